# revision 8
# baseline (speedup 1.0000x reference)
"""Trainium2 Bass kernel for nn_Attn_88759794139141.

Multi-head attention (B=2, S=2048, D=1024, H=16, K=64), eval mode.
Sharded tensor-parallel over heads: 8 cores x 2 heads each.

Per-core pipeline (all matmuls fp32r):
  1. QKV:   qkvT[feat, tok] = WqkvT_c.T @ xT   (feat-major: 2 heads x (q|k|v))
  2. V transpose: vT -> v_aug [tok-major, 64ch + ones column]
  3. Attention per (batch, q-half): scoresT[kpos, q] via head-pair packed
     K=64 matmuls -> exp on ScalarE (PSUM->SBUF) -> AV accumulation with
     ones-augmented V giving [o_unnorm | denom] -> normalize.
  4. Proj partial: outT[feat, tok] = WprojT_c.T @ o_pair  (summed on host)
"""

import numpy as np
from contextlib import ExitStack

import concourse.bass as bass
import concourse.tile as tile
import concourse.mybir as mybir
from concourse import bacc
from concourse.bass_utils import run_bass_kernel_spmd
from concourse.masks import make_identity

F32 = mybir.dt.float32
F32R = mybir.dt.float32r

B = 2
S = 2048
D = 1024
H = 16
KH = 64            # head dim
NC = 8             # cores
HPC = H // NC      # heads per core = 2
TOK = B * S        # 4096
FEAT = 3 * HPC * KH  # 384 qkv features per core
SCALE = 1.0 / 8.0  # 1/sqrt(64)

_CACHE = {}


def build_kernel(phases=4):
    nc = bacc.Bacc("TRN2", target_bir_lowering=False, debug=False)

    xT_d = nc.dram_tensor("xT", [D, TOK], F32R, kind="ExternalInput").ap()
    wqkvT_d = nc.dram_tensor("wqkvT", [D, FEAT], F32R, kind="ExternalInput").ap()
    bqkv_d = nc.dram_tensor("bqkv", [128, 3], F32, kind="ExternalInput").ap()
    wprojT_d = nc.dram_tensor("wprojT", [128, D], F32R, kind="ExternalInput").ap()
    outT_d = nc.dram_tensor("outT", [D, TOK], F32, kind="ExternalOutput").ap()

    with tile.TileContext(nc) as tc, ExitStack() as ctx:
        const = ctx.enter_context(tc.tile_pool(name="const", bufs=1))
        xpool = ctx.enter_context(tc.tile_pool(name="xpool", bufs=2))
        qkv = ctx.enter_context(tc.tile_pool(name="qkv", bufs=1))
        vpool = ctx.enter_context(tc.tile_pool(name="vpool", bufs=1))
        ppool = ctx.enter_context(tc.tile_pool(name="ppool", bufs=3))
        opool = ctx.enter_context(tc.tile_pool(name="opool", bufs=1))
        rdpool = ctx.enter_context(tc.tile_pool(name="rdpool", bufs=2))
        outp = ctx.enter_context(tc.tile_pool(name="outp", bufs=3))

        # constants
        wqkvT_sb = const.tile([128, 8, FEAT], F32R)
        nc.sync.dma_start(
            wqkvT_sb[:], wqkvT_d.rearrange("(k p) f -> p k f", p=128))
        bqkv_sb = const.tile([128, 3], F32)
        nc.sync.dma_start(bqkv_sb[:], bqkv_d[:])
        wprojT_sb = const.tile([128, D], F32R)
        nc.sync.dma_start(wprojT_sb[:], wprojT_d[:])
        ident_f32 = const.tile([128, 128], F32)
        make_identity(nc, ident_f32[:])
        ident = const.tile([128, 128], F32R)
        nc.vector.tensor_copy(ident[:], ident_f32[:])

        # persistent feature-major qkv
        qT = qkv.tile([128, TOK], F32R, tag="qT")
        kT = qkv.tile([128, TOK], F32R, tag="kT")
        vT = qkv.tile([128, TOK], F32R, tag="vT")
        dests = [qT, kT, vT]

        # token-major v with ones column: [128, head, tok_tile, 65]
        v_aug = vpool.tile([128, HPC, TOK // 128, KH + 1], F32R)
        ones_col = const.tile([128, 1], F32)
        nc.gpsimd.memset(ones_col[:], 1.0)
        nc.vector.tensor_copy(
            v_aug[:, :, :, KH:KH + 1],
            ones_col[:, None, None, :].to_broadcast((128, HPC, TOK // 128, 1)))

        # o_pair [ch(2 heads), batch, tok-in-batch]
        o_pair = opool.tile([128, B, S], F32R)

        xT_r = xT_d.rearrange("(k p) t -> p k t", p=128)

        with tc.tile_pool(name="ps_a", bufs=3, space="PSUM") as ps_a, \
             tc.tile_pool(name="ps_tr", bufs=4, space="PSUM") as ps_tr:
            # ---- Phase 1: QKV ----
            for blk in range(TOK // 512):
                xt = xpool.tile([128, 8, 512], F32R, tag="xt")
                nc.sync.dma_start(xt[:], xT_r[:, :, blk * 512:(blk + 1) * 512])
                for f in range(3):
                    ps = ps_a.tile([128, 512], F32, tag="qkv")
                    for k in range(8):
                        nc.tensor.matmul(
                            ps[:], wqkvT_sb[:, k, f * 128:(f + 1) * 128],
                            xt[:, k, :], start=(k == 0), stop=(k == 7))
                    nc.vector.tensor_add(
                        dests[f][:, blk * 512:(blk + 1) * 512], ps[:],
                        bqkv_sb[:, f, None].to_broadcast((128, 512)))

            # ---- Phase 2: transpose V to token-major ----
            for h in range(HPC if phases >= 2 else 0):
                hs = slice(h * KH, (h + 1) * KH)
                for t in range(TOK // 128):
                    pt = ps_tr.tile([128, KH], F32R, tag="tr")
                    nc.tensor.transpose(
                        pt[:], vT[hs, t * 128:(t + 1) * 128],
                        ident[hs, hs])
                    nc.vector.tensor_copy(v_aug[:, h, t, 0:KH], pt[:])

        # ---- Phase 3: attention ----
        with tc.tile_pool(name="ps_sc", bufs=1, space="PSUM") as ps_sc, \
             tc.tile_pool(name="ps_o", bufs=1, space="PSUM") as ps_o:
            QH = S // 2  # q-half = 1024
            for b in range(B if phases >= 3 else 0):
                for qh in range(2):
                    q0 = b * S + qh * QH  # global q offset
                    oT = [ps_o.tile([KH + 1, QH], F32, tag=f"o{h}",
                                    name=f"oT{h}")
                          for h in range(HPC)]
                    for j in range(S // 128):  # kpos tiles
                        kp0 = b * S + j * 128
                        sc = ps_sc.tile([128, 2 * QH], F32, tag="sc")
                        for h in range(HPC):
                            hs = slice(h * KH, (h + 1) * KH)
                            for qb in range(QH // 512):
                                nc.tensor.matmul(
                                    sc[:, h * QH + qb * 512:h * QH + (qb + 1) * 512],
                                    kT[hs, kp0:kp0 + 128],
                                    qT[hs, q0 + qb * 512:q0 + (qb + 1) * 512],
                                    start=True, stop=True,
                                    tile_position=(h * KH, 0))
                        p = ppool.tile([128, 2 * QH], F32R, tag="p")
                        nc.scalar.activation(
                            p[:], sc[:], mybir.ActivationFunctionType.Exp,
                            scale=SCALE)
                        for h in range(HPC):
                            for qb in range(QH // 512):
                                nc.tensor.matmul(
                                    oT[h][:, qb * 512:(qb + 1) * 512],
                                    v_aug[:, h, (b * S) // 128 + j, :],
                                    p[:, h * QH + qb * 512:h * QH + (qb + 1) * 512],
                                    start=(j == 0), stop=(j == S // 128 - 1))
                    # normalize: o = o_unnorm * (1/denom), assemble head pair
                    for h in range(HPC):
                        rd1 = rdpool.tile([1, QH], F32, tag="rd1")
                        nc.vector.reciprocal(rd1[:], oT[h][KH:KH + 1, :])
                        rd = rdpool.tile([KH, QH], F32, tag="rd")
                        nc.gpsimd.partition_broadcast(rd[:], rd1[:])
                        nc.vector.tensor_mul(
                            o_pair[h * KH:(h + 1) * KH, b, qh * QH:(qh + 1) * QH],
                            oT[h][0:KH, :], rd[:])

        # ---- Phase 4: projection ----
        outT_r = outT_d.rearrange("(f p) t -> p f t", p=128)
        with tc.tile_pool(name="ps_pr", bufs=4, space="PSUM") as ps_pr:
            if phases < 4:
                # dump qT/o_pair instead so outT has a writer
                dbg = qT if phases < 3 else o_pair.rearrange("p b s -> p (b s)")
                for tb in range(TOK // 512):
                    ot = outp.tile([128, 512], F32, tag="ot")
                    nc.vector.tensor_copy(ot[:], dbg[:, tb * 512:(tb + 1) * 512])
                    nc.sync.dma_start(
                        outT_r[:, 0, tb * 512:(tb + 1) * 512], ot[:])
            for b in range(B if phases >= 4 else 0):
                for tb in range(S // 512):
                    for f in range(D // 128):
                        pp = ps_pr.tile([128, 512], F32, tag="pp")
                        nc.tensor.matmul(
                            pp[:], wprojT_sb[:, f * 128:(f + 1) * 128],
                            o_pair[:, b, tb * 512:(tb + 1) * 512],
                            start=True, stop=True)
                        ot = outp.tile([128, 512], F32, tag="ot")
                        nc.vector.tensor_copy(ot[:], pp[:])
                        nc.sync.dma_start(
                            outT_r[:, f, b * S + tb * 512:b * S + (tb + 1) * 512],
                            ot[:])

    nc.compile()
    return nc


def _in_maps(x, Wqkv, bqkv, Wproj):
    xT = np.ascontiguousarray(x.reshape(TOK, D).T.astype(np.float32))
    maps = []
    for c in range(NC):
        rows = np.concatenate(
            [Wqkv[t * D + 128 * c: t * D + 128 * (c + 1)] for t in range(3)],
            axis=0)  # [384, 1024]
        wqkvT = np.ascontiguousarray(rows.T.astype(np.float32))
        brows = np.concatenate(
            [bqkv[t * D + 128 * c: t * D + 128 * (c + 1)] for t in range(3)])
        bq = np.ascontiguousarray(
            brows.reshape(3, 128).T.astype(np.float32))
        wprojT = np.ascontiguousarray(
            Wproj[:, 128 * c: 128 * (c + 1)].T.astype(np.float32))
        maps.append({"xT": xT, "wqkvT": wqkvT, "bqkv": bq, "wprojT": wprojT})
    return maps


def kernel(x, Wqkv, bqkv, Wproj, bproj, trace=False):
    x = np.asarray(x)
    Wqkv = np.asarray(Wqkv)
    bqkv = np.asarray(bqkv)
    Wproj = np.asarray(Wproj)
    bproj = np.asarray(bproj)

    if "nc" not in _CACHE:
        _CACHE["nc"] = build_kernel()
    nc = _CACHE["nc"]

    res = run_bass_kernel_spmd(nc, _in_maps(x, Wqkv, bqkv, Wproj),
                               core_ids=list(range(NC)), trace=trace)
    acc = np.zeros((D, TOK), dtype=np.float64)
    for c in range(NC):
        acc += res.results[c]["outT"]
    out = acc.T + bproj.astype(np.float64)
    _CACHE["last_result"] = res
    return out.reshape(B, S, D).astype(np.float32)
